# revision 6
# baseline (speedup 1.0000x reference)
"""Distributed CAP-memory loss kernel for 8 TRN2 NeuronCores (fp8 version).

Problem (see reference): given unit-norm features [B=256, D=2048] and a
memory bank [6, 2000, 2048], compute
  loss = sum_cam mean_cam(per-camera proxy CE)
       + 0.5 * sum_cam mean_cam(assoc loss over 6 positives + 50 hard negatives)

Distribution strategy (column/class sharding, interleaved):
  The 12000 memory rows are split so core k owns columns
  {j*2000 + k*250 + r : j in [0,6), r in [0,250)} -- an identical 250-wide
  slice of every camera block, so all 8 cores run the same SPMD program on
  6 groups x 250 columns each.

Device program (per core):
  * inputs are pre-scaled by S=32 and quantized to fp8 e4m3 on the host
    (sims error ~1e-3 vs sims std 0.026; loss rel err ~7e-5, tol 2e-2)
  * sims_local = feats @ memT_local via DoubleRow fp8 matmuls (2 ko-tiles
    per instruction, 2x PE rate), f32 PSUM, one [128,250] accumulation
    region per (group, batch-tile)
  * per-group top-8 (DVE MAX8 straight from PSUM) -> 48 candidates/core
  * per-group sum(exp(20*sims)) (ACT exp straight from PSUM, accum_out)

The host merges the per-core stats ([256, 54] each): removes positives
from the candidate lists (matched against host-recomputed quantized
positive values), takes the global top-50 with an exactness certificate
and an exact (quantized) fallback, log-sum-exp combines with the exact
f64 positives, segment sums -> scalar loss.

memT is host-packed group-major ([P, group, ko, W]) so every DMA piece is
a fully contiguous [P, n] slice with 2000-byte runs per partition,
streamed in PE-consumption order over the sync/gpsimd/vector queues
(scalar stays free so its exp-table load never delays input streaming).
"""

import os
import sys
import types

import numpy as np

# ---------------------------------------------------------------- constants
B = 256          # batch
D = 2048         # feature dim
NCAMS = 6
C = 2000         # classes per camera
NG = NCAMS * C   # 12000 global columns
M = 8            # cores
W = C // M       # 250: per-core slice width inside each camera block
NL = NCAMS * W   # 1500 local columns per core
P = 128          # partitions
KO = D // P      # 16 contraction chunks
KP = KO // 2     # 8 DoubleRow ko-pairs
BT = B // P      # 2 batch tiles
BETA = 0.05
INV_BETA = 1.0 / BETA  # 20.0
BG_KNN = 50
NK = 8           # top-8 candidates per 250-wide group
NCAND = NCAMS * NK      # 48 candidates per core
OUTC = NCAND + NCAMS    # 48 topk | 6 sumexp
FP8_SCALE = 32.0        # host pre-scale before e4m3 quantization
S2 = FP8_SCALE * FP8_SCALE      # sims come back scaled by this
POS_TOL_SCALED = 0.1    # |device - host| match tolerance, scaled units

LAST_EXEC_NS = None
LAST_OUTS = None
FALLBACK_COUNT = 0
_NC_CACHE = {}


def _install_axon_ntff_hook():
    """The agent image's antenv lacks axon_hooks; synthesize it so
    run_bass_kernel_spmd(trace=True) can capture NTFF profiles."""
    if "antenv.axon_hooks" in sys.modules:
        return
    mod = types.ModuleType("antenv.axon_hooks")
    state = {"hook": None}
    mod.set_axon_ntff_profile_hook = lambda h: state.__setitem__("hook", h)
    mod.get_axon_ntff_profile_hook = lambda: state["hook"]
    sys.modules["antenv.axon_hooks"] = mod
    try:
        import antenv

        antenv.axon_hooks = mod
    except Exception:
        pass
    try:
        from trn_agent_boot.trn_boot import _ntff_profile_via_ctypes

        hook = _ntff_profile_via_ctypes("/opt/axon/libaxon_pjrt.so")
        if hook is not None:
            mod.set_axon_ntff_profile_hook(hook)
    except Exception:
        pass


def build_nc():
    """Build + compile the single SPMD Bass program shared by all 8 cores."""
    import concourse.bacc as bacc
    import concourse.mybir as mybir
    import concourse.tile as tile

    f32 = mybir.dt.float32
    fp8 = mybir.dt.float8e4
    AF = mybir.ActivationFunctionType
    DR = mybir.MatmulPerfMode.DoubleRow
    A = mybir.AluOpType
    AX = mybir.AxisListType

    nc = bacc.Bacc(
        "TRN2",
        target_bir_lowering=False,
        debug=False,
        enable_asserts=False,
        num_devices=M,
    )

    featsT_d = nc.dram_tensor("featsT", [P, KO * B], fp8, kind="ExternalInput")
    memT_d = nc.dram_tensor("memT", [P, NCAMS * KO * W], fp8, kind="ExternalInput")
    out_d = nc.dram_tensor("out", [B, OUTC], f32, kind="ExternalOutput")

    with tile.TileContext(nc) as tc:
        with (
            tc.tile_pool(name="big", bufs=1) as big,
            tc.tile_pool(name="work", bufs=BT) as work,
            tc.tile_pool(name="scr", bufs=4) as scr,
            tc.tile_pool(name="psum", bufs=NCAMS, space="PSUM") as psum,
        ):
            featsT_sb = big.tile([P, KO * B], fp8)
            memT_sb = big.tile([P, NCAMS * KO * W], fp8)
            # scalar is listed last: its exp-table load delays its stream
            # start ~1.3us, so it only gets later-deadline pieces
            queues = [nc.sync, nc.gpsimd, nc.scalar]

            mw_d = memT_d[:].rearrange("p (g kw) -> p g kw", g=NCAMS)
            mw_s = memT_sb[:].rearrange("p (g kw) -> p g kw", g=NCAMS)

            def feats_piece(qi, klo, khi):
                fsl = slice(klo * B, khi * B)
                queues[qi].dma_start(featsT_sb[:, fsl], featsT_d[:, fsl])

            def set_piece(qi, s, klo, khi):
                # all 3 groups of set s, ko range [klo,khi): 3 runs of
                # (khi-klo)*250 contiguous bytes per partition
                gsl = slice(3 * s, 3 * s + 3)
                ksl = slice(klo * W, khi * W)
                queues[qi].dma_start(mw_s[:, gsl, ksl], mw_d[:, gsl, ksl])

            # issue in PE-consumption (deadline) order.  setA ko-slabs are
            # fine-grained (2,2,4,4,4 ko) so the PE starts as early as
            # possible; setB uses ko-quarters.  Queues: sync starts ~1us
            # before gpsimd, scalar ~0.5us later (exp-table load first).
            feats_piece(0, 0, 4)      # sync:   f0 -> a1 -> a3 -> b1 -> out0
            set_piece(1, 0, 0, 2)     # gpsimd: a0 -> a2 -> b0 -> b3 -> out1
            feats_piece(2, 4, 10)     # scalar: f1 -> f2 -> a4 -> b2
            set_piece(0, 0, 2, 4)
            feats_piece(2, 10, 16)
            set_piece(1, 0, 4, 8)
            set_piece(0, 0, 8, 12)
            set_piece(2, 0, 12, 16)
            set_piece(1, 1, 0, 4)
            set_piece(0, 1, 4, 8)
            set_piece(2, 1, 8, 12)
            set_piece(1, 1, 12, 16)

            outs = [
                work.tile([P, OUTC], f32, tag="outs", name=f"outs{b}")
                for b in range(BT)
            ]
            # 6 bank-sized [128, 500] f32 PSUM tiles; (g, bt) accumulates
            # into half of tile g so two groups never straddle a bank
            pstiles = [
                psum.tile([P, 2 * W], f32, tag="ps", name=f"ps{g}")
                for g in range(NCAMS)
            ]

            fv = featsT_sb[:].rearrange("p (ko b) -> p ko b", b=B)
            mv = memT_sb[:].rearrange("p (gk w) -> p gk w", w=W)

            def epilogue(g, bt):
                ps = pstiles[g][:, bt * W : (bt + 1) * W]
                # top-8 of this group's 250 columns, straight from PSUM
                nc.vector.max(out=outs[bt][:, g * NK : (g + 1) * NK], in_=ps)
                # per-group sum(exp(sims/beta)); scaled sims * (20/1024)
                et = scr.tile([P, W], f32, tag="exp")
                nc.scalar.activation(
                    et[:],
                    ps,
                    AF.Exp,
                    scale=INV_BETA / S2,
                    accum_out=outs[bt][:, NCAND + g : NCAND + g + 1],
                )

            # group triples share one stationary (lhsT) load per (kp, bt):
            # consecutive matmuls with an identical weights AP skip the
            # ~107ns PE weight reload (DR matmuls cannot shadow-load)
            for s in range(2):
                for bt in range(BT):
                    for kp in range(KP):
                        lhsT = fv[:, 2 * kp : 2 * kp + 2, bt * P : (bt + 1) * P]
                        for gi in range(3):
                            g = 3 * s + gi
                            nc.tensor.matmul(
                                pstiles[g][:, bt * W : (bt + 1) * W],
                                lhsT,
                                mv[:, g * KO + 2 * kp : g * KO + 2 * kp + 2, :],
                                start=(kp == 0),
                                stop=(kp == KP - 1),
                                perf_mode=DR,
                            )
                    for gi in range(3):
                        epilogue(3 * s + gi, bt)

            nc.sync.dma_start(out_d[0:P, :], outs[0][:])
            nc.gpsimd.dma_start(out_d[P : 2 * P, :], outs[1][:])

    nc.compile()
    return nc


def get_nc():
    if "nc" not in _NC_CACHE:
        _NC_CACHE["nc"] = build_nc()
    return _NC_CACHE["nc"]


def _fp8_np_dtype():
    import ml_dtypes

    return np.dtype(ml_dtypes.float8_e4m3)


def quantize(arr: np.ndarray) -> np.ndarray:
    """Scale by FP8_SCALE and round to e4m3."""
    return (np.asarray(arr, np.float32) * FP8_SCALE).astype(_fp8_np_dtype())


def shard_cols(k: int) -> np.ndarray:
    """Global memory-bank columns owned by core k."""
    return (
        np.arange(NCAMS)[:, None] * C + k * W + np.arange(W)[None, :]
    ).reshape(-1)


def pack_featsT(fq: np.ndarray) -> np.ndarray:
    """Quantized [B, D] -> [P, KO*B] with row p holding feats.T[ko*128+p, :]."""
    arr = fq.T.reshape(KO, P, B).transpose(1, 0, 2).reshape(P, KO * B)
    return np.ascontiguousarray(arr)


def pack_memT(mem_flat_q: np.ndarray, cols: np.ndarray) -> np.ndarray:
    """Quantized [NG, D] -> [P, NCAMS*KO*W] group-major for this core."""
    arr = (
        mem_flat_q[cols]
        .T.reshape(KO, P, NCAMS, W)
        .transpose(1, 2, 0, 3)
        .reshape(P, NCAMS * KO * W)
    )
    return np.ascontiguousarray(arr)


def _loss_from_parts(pos_logits, lse_block, top50, cams):
    rows = np.arange(B)
    ce = lse_block[rows, cams] - pos_logits[rows, cams]
    logits = np.concatenate([pos_logits, INV_BETA * top50], axis=1)
    mx = logits.max(axis=1, keepdims=True)
    lse56 = mx[:, 0] + np.log(np.exp(logits - mx).sum(axis=1))
    assoc = lse56 - pos_logits.sum(axis=1) / NCAMS

    counts = np.bincount(cams, minlength=NCAMS).astype(np.float64)
    ce_sum = np.bincount(cams, weights=ce, minlength=NCAMS)
    as_sum = np.bincount(cams, weights=assoc, minlength=NCAMS)
    safe = np.maximum(counts, 1.0)
    present = counts > 0
    return np.sum(np.where(present, ce_sum / safe, 0.0)) + np.sum(
        np.where(present, 0.5 * as_sum / safe, 0.0)
    )


def host_combine(outs, fq, mq, features, memory, cams, labels):
    """outs: [M, B, OUTC] device results (candidate values scaled by S2)."""
    global FALLBACK_COUNT
    rows = np.arange(B)
    cand = outs[:, :, :NCAND].astype(np.float64)  # [M, B, 48] scaled
    sexp = outs[:, :, NCAND:].astype(np.float64)  # [M, B, 6]

    s_block = sexp.sum(axis=0)   # [B, 6] sum(exp(20*sims)) per camera block
    lse_block = np.log(s_block)  # logsumexp of own-camera logits

    # exact positives for the loss; quantized positives for list matching
    feats64 = np.asarray(features, np.float64)
    pos_vals = np.einsum(
        "bd,jbd->bj",
        feats64,
        np.asarray(memory, np.float64)[:, labels, :],
        optimize=True,
    )  # [B, 6] exact, unscaled
    pos_q = np.einsum(
        "bd,jbd->bj",
        fq.astype(np.float64),
        mq.astype(np.float64)[:, labels, :],
        optimize=True,
    )  # [B, 6] device-accurate, scaled

    # [B, M*NCAMS, 8] per-(core,group) candidate lists
    percl = cand.transpose(1, 0, 2).reshape(B, M * NCAMS, NK).copy()
    cmin_raw = percl.min(axis=2)  # pre-drop floor per (core,group)

    # Remove positives.  Positive (i, j) can only appear on core
    # labels[i]//W in group j; drop the closest value within tolerance
    # (device and host compute the same quantized dot product, so a
    # present positive matches to ~1e-4 scaled; candidate spacing is ~1).
    own_core = labels // W  # [B]
    for j in range(NCAMS):
        cl = own_core * NCAMS + j  # [B] list index
        lists = percl[rows, cl]    # [B, 8] (fancy idx: copy)
        diff = np.abs(lists - pos_q[:, j : j + 1])
        am = diff.argmin(axis=1)
        hit = diff[rows, am] < POS_TOL_SCALED
        lists[hit, am[hit]] = -np.inf
        percl[rows, cl] = lists

    flat = percl.reshape(B, -1)
    top50s = -np.partition(-flat, BG_KNN - 1, axis=1)[:, :BG_KNN]  # scaled
    t50 = top50s[:, BG_KNN - 1]  # [B] 50th largest of the union, scaled

    # Exactness certificate: every (core,group)'s smallest extracted
    # candidate must lie strictly below the union's 50th value, proving no
    # unseen value could reach the global top-50.
    bad = (cmin_raw >= t50[:, None]).any(axis=1)
    if bad.any():
        # Exact fallback for insufficient rows: recompute (quantized, to
        # stay consistent with the device values) on the host.
        FALLBACK_COUNT += int(bad.sum())
        mem_flat_q = mq.reshape(NG, D).astype(np.float32)
        idx = np.nonzero(bad)[0]
        sims = fq.astype(np.float32)[idx] @ mem_flat_q.T  # scaled
        colsg = np.arange(NG)
        for p, i in enumerate(idx):
            row = sims[p].astype(np.float64)
            row[colsg % C == labels[i]] = -np.inf
            top50s[i] = -np.sort(-row)[:BG_KNN]

    top50 = top50s / S2
    return np.float32(
        _loss_from_parts(INV_BETA * pos_vals, lse_block, top50, cams)
    )


def kernel(features, memory, cams, labels, trace: bool = None):
    global LAST_EXEC_NS, LAST_OUTS
    _install_axon_ntff_hook()
    from concourse.bass_utils import run_bass_kernel_spmd

    features = np.asarray(features, dtype=np.float32)
    memory = np.asarray(memory, dtype=np.float32)
    cams = np.asarray(cams).astype(np.int64)
    labels = np.asarray(labels).astype(np.int64)

    nc = get_nc()

    fq = quantize(features)              # [B, D] fp8, scaled
    mq = quantize(memory)                # [6, C, D] fp8, scaled
    mem_flat_q = mq.reshape(NG, D)
    featsT = pack_featsT(fq)
    in_maps = [
        {"featsT": featsT, "memT": pack_memT(mem_flat_q, shard_cols(k))}
        for k in range(M)
    ]

    if trace is None:
        trace = os.environ.get("CAP_TRACE", "1") == "1"
    res = run_bass_kernel_spmd(
        nc, in_maps, core_ids=list(range(M)), trace=trace
    )
    if res.exec_time_ns is not None:
        LAST_EXEC_NS = res.exec_time_ns

    outs = np.stack([r["out"] for r in res.results])  # [M, B, OUTC]
    LAST_OUTS = outs
    return np.asarray(
        host_combine(outs, fq, mq, features, memory, cams, labels),
        dtype=np.float32,
    )


# ------------------------------------------------------------------ helpers
def expected_core_out(features, memory, k: int) -> np.ndarray:
    """Numpy model of what core k's device program should output [B, OUTC]."""
    fq = quantize(features).astype(np.float32)
    mem_flat_q = quantize(memory).reshape(NG, D).astype(np.float32)
    cols = shard_cols(k)
    sims = fq @ mem_flat_q[cols].T  # [B, NL] scaled
    out = np.zeros((B, OUTC), np.float32)
    for g in range(NCAMS):
        blk = sims[:, g * W : (g + 1) * W]
        srt = -np.sort(-blk, axis=1)
        out[:, g * NK : (g + 1) * NK] = srt[:, :NK]
        out[:, NCAND + g] = np.exp(
            (INV_BETA / S2) * blk.astype(np.float64)
        ).sum(axis=1)
    return out


# revision 7
# speedup vs baseline: 1.0720x; 1.0720x over previous
"""Distributed CAP-memory loss kernel for 8 TRN2 NeuronCores (fp8 version).

Problem (see reference): given unit-norm features [B=256, D=2048] and a
memory bank [6, 2000, 2048], compute
  loss = sum_cam mean_cam(per-camera proxy CE)
       + 0.5 * sum_cam mean_cam(assoc loss over 6 positives + 50 hard negatives)

Distribution strategy (column/class sharding, interleaved):
  The 12000 memory rows are split so core k owns columns
  {j*2000 + k*250 + r : j in [0,6), r in [0,250)} -- an identical 250-wide
  slice of every camera block, so all 8 cores run the same SPMD program on
  6 groups x 250 columns each.

Device program (per core):
  * inputs are pre-scaled by S=32 and quantized to fp8 e4m3 on the host
    (sims error ~1e-3 vs sims std 0.026; loss rel err ~7e-5, tol 2e-2)
  * sims_local = feats @ memT_local via DoubleRow fp8 matmuls (2 ko-tiles
    per instruction, 2x PE rate), f32 PSUM, one [128,250] accumulation
    region per (group, batch-tile)
  * per-group top-8 (DVE MAX8 straight from PSUM) -> 48 candidates/core
  * per-group sum(exp(20*sims)) (ACT exp straight from PSUM, accum_out)

The host merges the per-core stats ([256, 54] each): removes positives
from the candidate lists (matched against host-recomputed quantized
positive values), takes the global top-50 with an exactness certificate
and an exact (quantized) fallback, log-sum-exp combines with the exact
f64 positives, segment sums -> scalar loss.

memT is host-packed group-major ([P, group, ko, W]) so every DMA piece is
a fully contiguous [P, n] slice with 2000-byte runs per partition,
streamed in PE-consumption order over the sync/gpsimd/vector queues
(scalar stays free so its exp-table load never delays input streaming).
"""

import os
import sys
import types

import numpy as np

# ---------------------------------------------------------------- constants
B = 256          # batch
D = 2048         # feature dim
NCAMS = 6
C = 2000         # classes per camera
NG = NCAMS * C   # 12000 global columns
M = 8            # cores
W = C // M       # 250: per-core slice width inside each camera block
NL = NCAMS * W   # 1500 local columns per core
P = 128          # partitions
KO = D // P      # 16 contraction chunks
KP = KO // 2     # 8 DoubleRow ko-pairs
BT = B // P      # 2 batch tiles
BETA = 0.05
INV_BETA = 1.0 / BETA  # 20.0
BG_KNN = 50
NK = 8           # top-8 candidates per 250-wide group
NCAND = NCAMS * NK      # 48 candidates per core
OUTC = NCAND + NCAMS    # 48 topk | 6 sumexp
FP8_SCALE = 32.0        # host pre-scale before e4m3 quantization
S2 = FP8_SCALE * FP8_SCALE      # sims come back scaled by this
POS_TOL_SCALED = 0.1    # |device - host| match tolerance, scaled units

LAST_EXEC_NS = None
LAST_OUTS = None
FALLBACK_COUNT = 0
_NC_CACHE = {}


def _install_axon_ntff_hook():
    """The agent image's antenv lacks axon_hooks; synthesize it so
    run_bass_kernel_spmd(trace=True) can capture NTFF profiles."""
    if "antenv.axon_hooks" in sys.modules:
        return
    mod = types.ModuleType("antenv.axon_hooks")
    state = {"hook": None}
    mod.set_axon_ntff_profile_hook = lambda h: state.__setitem__("hook", h)
    mod.get_axon_ntff_profile_hook = lambda: state["hook"]
    sys.modules["antenv.axon_hooks"] = mod
    try:
        import antenv

        antenv.axon_hooks = mod
    except Exception:
        pass
    try:
        from trn_agent_boot.trn_boot import _ntff_profile_via_ctypes

        hook = _ntff_profile_via_ctypes("/opt/axon/libaxon_pjrt.so")
        if hook is not None:
            mod.set_axon_ntff_profile_hook(hook)
    except Exception:
        pass


def build_nc():
    """Build + compile the single SPMD Bass program shared by all 8 cores."""
    import concourse.bacc as bacc
    import concourse.mybir as mybir
    import concourse.tile as tile

    f32 = mybir.dt.float32
    fp8 = mybir.dt.float8e4
    AF = mybir.ActivationFunctionType
    DR = mybir.MatmulPerfMode.DoubleRow
    A = mybir.AluOpType
    AX = mybir.AxisListType

    nc = bacc.Bacc(
        "TRN2",
        target_bir_lowering=False,
        debug=False,
        enable_asserts=False,
        num_devices=M,
    )

    featsT_d = nc.dram_tensor("featsT", [P, KO * B], fp8, kind="ExternalInput")
    memT_d = nc.dram_tensor("memT", [P, NCAMS * KO * W], fp8, kind="ExternalInput")
    out_d = nc.dram_tensor("out", [B, OUTC], f32, kind="ExternalOutput")

    with tile.TileContext(nc) as tc:
        with (
            tc.tile_pool(name="big", bufs=1) as big,
            tc.tile_pool(name="work", bufs=BT) as work,
            tc.tile_pool(name="scr", bufs=4) as scr,
            tc.tile_pool(name="psum", bufs=NCAMS, space="PSUM") as psum,
        ):
            featsT_sb = big.tile([P, KO * B], fp8)
            memT_sb = big.tile([P, NCAMS * KO * W], fp8)
            # scalar is listed last: its exp-table load delays its stream
            # start ~1.3us, so it only gets later-deadline pieces
            queues = [nc.sync, nc.gpsimd, nc.scalar]

            mw_d = memT_d[:].rearrange("p (g kw) -> p g kw", g=NCAMS)
            mw_s = memT_sb[:].rearrange("p (g kw) -> p g kw", g=NCAMS)

            def feats_piece(qi, klo, khi):
                fsl = slice(klo * B, khi * B)
                queues[qi].dma_start(featsT_sb[:, fsl], featsT_d[:, fsl])

            def set_piece(qi, s, klo, khi):
                # all 3 groups of set s, ko range [klo,khi): 3 runs of
                # (khi-klo)*250 contiguous bytes per partition
                gsl = slice(3 * s, 3 * s + 3)
                ksl = slice(klo * W, khi * W)
                queues[qi].dma_start(mw_s[:, gsl, ksl], mw_d[:, gsl, ksl])

            # issue in PE-consumption (deadline) order.  All mem pieces are
            # ko-pair slabs (192KB, matching the matmul's DR gating
            # granularity) so the PE starts on the first 192KB and never
            # waits on coarse pieces.  feats stages on sync (earliest
            # queue); slabs alternate gpsimd (fastest) / scalar (starts
            # ~1.3us late behind its exp-table load).
            feats_piece(0, 0, 2)
            set_piece(1, 0, 0, 2)     # a0
            feats_piece(0, 2, 4)
            set_piece(1, 0, 2, 4)     # a1
            set_piece(2, 0, 6, 8)     # a3
            feats_piece(0, 4, 8)
            set_piece(1, 0, 4, 6)     # a2
            set_piece(2, 0, 10, 12)   # a5
            feats_piece(0, 8, 12)
            set_piece(1, 0, 8, 10)    # a4
            feats_piece(0, 12, 16)
            set_piece(1, 0, 12, 14)   # a6
            set_piece(2, 0, 14, 16)   # a7
            set_piece(1, 1, 0, 2)     # b0
            set_piece(2, 1, 2, 4)     # b1
            set_piece(0, 1, 4, 6)     # b2
            set_piece(1, 1, 6, 8)     # b3
            set_piece(2, 1, 8, 10)    # b4
            set_piece(0, 1, 10, 12)   # b5
            set_piece(1, 1, 12, 14)   # b6
            set_piece(2, 1, 14, 16)   # b7

            outs = [
                work.tile([P, OUTC], f32, tag="outs", name=f"outs{b}")
                for b in range(BT)
            ]
            # 6 bank-sized [128, 500] f32 PSUM tiles; (g, bt) accumulates
            # into half of tile g so two groups never straddle a bank
            pstiles = [
                psum.tile([P, 2 * W], f32, tag="ps", name=f"ps{g}")
                for g in range(NCAMS)
            ]

            fv = featsT_sb[:].rearrange("p (ko b) -> p ko b", b=B)
            mv = memT_sb[:].rearrange("p (gk w) -> p gk w", w=W)

            def epilogue(g, bt):
                ps = pstiles[g][:, bt * W : (bt + 1) * W]
                # top-8 of this group's 250 columns, straight from PSUM
                nc.vector.max(out=outs[bt][:, g * NK : (g + 1) * NK], in_=ps)
                # per-group sum(exp(sims/beta)); scaled sims * (20/1024)
                et = scr.tile([P, W], f32, tag="exp")
                nc.scalar.activation(
                    et[:],
                    ps,
                    AF.Exp,
                    scale=INV_BETA / S2,
                    accum_out=outs[bt][:, NCAND + g : NCAND + g + 1],
                )

            # group triples share one stationary (lhsT) load per (kp, bt):
            # consecutive matmuls with an identical weights AP skip the
            # ~107ns PE weight reload (DR matmuls cannot shadow-load)
            for s in range(2):
                for bt in range(BT):
                    for kp in range(KP):
                        lhsT = fv[:, 2 * kp : 2 * kp + 2, bt * P : (bt + 1) * P]
                        for gi in range(3):
                            g = 3 * s + gi
                            nc.tensor.matmul(
                                pstiles[g][:, bt * W : (bt + 1) * W],
                                lhsT,
                                mv[:, g * KO + 2 * kp : g * KO + 2 * kp + 2, :],
                                start=(kp == 0),
                                stop=(kp == KP - 1),
                                perf_mode=DR,
                            )
                    for gi in range(3):
                        epilogue(3 * s + gi, bt)

            nc.sync.dma_start(out_d[0:P, :], outs[0][:])
            nc.gpsimd.dma_start(out_d[P : 2 * P, :], outs[1][:])

    nc.compile()
    return nc


def get_nc():
    if "nc" not in _NC_CACHE:
        _NC_CACHE["nc"] = build_nc()
    return _NC_CACHE["nc"]


def _fp8_np_dtype():
    import ml_dtypes

    return np.dtype(ml_dtypes.float8_e4m3)


def quantize(arr: np.ndarray) -> np.ndarray:
    """Scale by FP8_SCALE and round to e4m3."""
    return (np.asarray(arr, np.float32) * FP8_SCALE).astype(_fp8_np_dtype())


def shard_cols(k: int) -> np.ndarray:
    """Global memory-bank columns owned by core k."""
    return (
        np.arange(NCAMS)[:, None] * C + k * W + np.arange(W)[None, :]
    ).reshape(-1)


def pack_featsT(fq: np.ndarray) -> np.ndarray:
    """Quantized [B, D] -> [P, KO*B] with row p holding feats.T[ko*128+p, :]."""
    arr = fq.T.reshape(KO, P, B).transpose(1, 0, 2).reshape(P, KO * B)
    return np.ascontiguousarray(arr)


def pack_memT(mem_flat_q: np.ndarray, cols: np.ndarray) -> np.ndarray:
    """Quantized [NG, D] -> [P, NCAMS*KO*W] group-major for this core."""
    arr = (
        mem_flat_q[cols]
        .T.reshape(KO, P, NCAMS, W)
        .transpose(1, 2, 0, 3)
        .reshape(P, NCAMS * KO * W)
    )
    return np.ascontiguousarray(arr)


def _loss_from_parts(pos_logits, lse_block, top50, cams):
    rows = np.arange(B)
    ce = lse_block[rows, cams] - pos_logits[rows, cams]
    logits = np.concatenate([pos_logits, INV_BETA * top50], axis=1)
    mx = logits.max(axis=1, keepdims=True)
    lse56 = mx[:, 0] + np.log(np.exp(logits - mx).sum(axis=1))
    assoc = lse56 - pos_logits.sum(axis=1) / NCAMS

    counts = np.bincount(cams, minlength=NCAMS).astype(np.float64)
    ce_sum = np.bincount(cams, weights=ce, minlength=NCAMS)
    as_sum = np.bincount(cams, weights=assoc, minlength=NCAMS)
    safe = np.maximum(counts, 1.0)
    present = counts > 0
    return np.sum(np.where(present, ce_sum / safe, 0.0)) + np.sum(
        np.where(present, 0.5 * as_sum / safe, 0.0)
    )


def host_combine(outs, fq, mq, features, memory, cams, labels):
    """outs: [M, B, OUTC] device results (candidate values scaled by S2)."""
    global FALLBACK_COUNT
    rows = np.arange(B)
    cand = outs[:, :, :NCAND].astype(np.float64)  # [M, B, 48] scaled
    sexp = outs[:, :, NCAND:].astype(np.float64)  # [M, B, 6]

    s_block = sexp.sum(axis=0)   # [B, 6] sum(exp(20*sims)) per camera block
    lse_block = np.log(s_block)  # logsumexp of own-camera logits

    # exact positives for the loss; quantized positives for list matching
    feats64 = np.asarray(features, np.float64)
    pos_vals = np.einsum(
        "bd,jbd->bj",
        feats64,
        np.asarray(memory, np.float64)[:, labels, :],
        optimize=True,
    )  # [B, 6] exact, unscaled
    pos_q = np.einsum(
        "bd,jbd->bj",
        fq.astype(np.float64),
        mq.astype(np.float64)[:, labels, :],
        optimize=True,
    )  # [B, 6] device-accurate, scaled

    # [B, M*NCAMS, 8] per-(core,group) candidate lists
    percl = cand.transpose(1, 0, 2).reshape(B, M * NCAMS, NK).copy()
    cmin_raw = percl.min(axis=2)  # pre-drop floor per (core,group)

    # Remove positives.  Positive (i, j) can only appear on core
    # labels[i]//W in group j; drop the closest value within tolerance
    # (device and host compute the same quantized dot product, so a
    # present positive matches to ~1e-4 scaled; candidate spacing is ~1).
    own_core = labels // W  # [B]
    for j in range(NCAMS):
        cl = own_core * NCAMS + j  # [B] list index
        lists = percl[rows, cl]    # [B, 8] (fancy idx: copy)
        diff = np.abs(lists - pos_q[:, j : j + 1])
        am = diff.argmin(axis=1)
        hit = diff[rows, am] < POS_TOL_SCALED
        lists[hit, am[hit]] = -np.inf
        percl[rows, cl] = lists

    flat = percl.reshape(B, -1)
    top50s = -np.partition(-flat, BG_KNN - 1, axis=1)[:, :BG_KNN]  # scaled
    t50 = top50s[:, BG_KNN - 1]  # [B] 50th largest of the union, scaled

    # Exactness certificate: every (core,group)'s smallest extracted
    # candidate must lie strictly below the union's 50th value, proving no
    # unseen value could reach the global top-50.
    bad = (cmin_raw >= t50[:, None]).any(axis=1)
    if bad.any():
        # Exact fallback for insufficient rows: recompute (quantized, to
        # stay consistent with the device values) on the host.
        FALLBACK_COUNT += int(bad.sum())
        mem_flat_q = mq.reshape(NG, D).astype(np.float32)
        idx = np.nonzero(bad)[0]
        sims = fq.astype(np.float32)[idx] @ mem_flat_q.T  # scaled
        colsg = np.arange(NG)
        for p, i in enumerate(idx):
            row = sims[p].astype(np.float64)
            row[colsg % C == labels[i]] = -np.inf
            top50s[i] = -np.sort(-row)[:BG_KNN]

    top50 = top50s / S2
    return np.float32(
        _loss_from_parts(INV_BETA * pos_vals, lse_block, top50, cams)
    )


def kernel(features, memory, cams, labels, trace: bool = None):
    global LAST_EXEC_NS, LAST_OUTS
    _install_axon_ntff_hook()
    from concourse.bass_utils import run_bass_kernel_spmd

    features = np.asarray(features, dtype=np.float32)
    memory = np.asarray(memory, dtype=np.float32)
    cams = np.asarray(cams).astype(np.int64)
    labels = np.asarray(labels).astype(np.int64)

    nc = get_nc()

    fq = quantize(features)              # [B, D] fp8, scaled
    mq = quantize(memory)                # [6, C, D] fp8, scaled
    mem_flat_q = mq.reshape(NG, D)
    featsT = pack_featsT(fq)
    in_maps = [
        {"featsT": featsT, "memT": pack_memT(mem_flat_q, shard_cols(k))}
        for k in range(M)
    ]

    if trace is None:
        trace = os.environ.get("CAP_TRACE", "1") == "1"
    res = run_bass_kernel_spmd(
        nc, in_maps, core_ids=list(range(M)), trace=trace
    )
    if res.exec_time_ns is not None:
        LAST_EXEC_NS = res.exec_time_ns

    outs = np.stack([r["out"] for r in res.results])  # [M, B, OUTC]
    LAST_OUTS = outs
    return np.asarray(
        host_combine(outs, fq, mq, features, memory, cams, labels),
        dtype=np.float32,
    )


# ------------------------------------------------------------------ helpers
def expected_core_out(features, memory, k: int) -> np.ndarray:
    """Numpy model of what core k's device program should output [B, OUTC]."""
    fq = quantize(features).astype(np.float32)
    mem_flat_q = quantize(memory).reshape(NG, D).astype(np.float32)
    cols = shard_cols(k)
    sims = fq @ mem_flat_q[cols].T  # [B, NL] scaled
    out = np.zeros((B, OUTC), np.float32)
    for g in range(NCAMS):
        blk = sims[:, g * W : (g + 1) * W]
        srt = -np.sort(-blk, axis=1)
        out[:, g * NK : (g + 1) * NK] = srt[:, :NK]
        out[:, NCAND + g] = np.exp(
            (INV_BETA / S2) * blk.astype(np.float64)
        ).sum(axis=1)
    return out
